# revision 20
# baseline (speedup 1.0000x reference)
"""Multi-head causal attention with RoPE on 8 TRN2 NeuronCores.

Problem: x[2,2048,2048] @ {Wq,Wk,Wv}ᵀ -> 16-head causal attention with RoPE
-> @ Woᵀ.  Sharding: core i handles batch i//4 and head-group i%4 (4 heads,
512 of the 2048 projection channels).  Wq/Wk/Wv are row-sliced, Wo is
column-sliced; each core emits a partial yᵀ and the host sums the 4 partials
per batch (the tensor-parallel all-reduce done at unshard time).

Device-side layout (all matmul operands bf16, fp32 PSUM accumulate):
  - host passes xᵀ[h,s] and Wᵀ[h,o] so every matmul contracts over the
    partition dim with zero on-chip transposes
  - scores are computed transposed, Sᵀ[k,q] = Kᵀ-chunkᵀ @ Qᵀ, so the exp'd
    attention chunk is directly the lhsT/rhs the PV matmul needs
  - no max-subtraction: scores are ~N(0,1) after the 1/sqrt(128) scale (fused
    into the ACT exp), so exp can't overflow fp32
  - softmax denominator: DVE accumulates the exp'd chunk tiles over the
    key-chunk axis (bf16, 2x mode) so the PE only sees a single 512-column
    all-ones matmul per (head, qblock) -- sum over the 128 key partitions
    with M=128, which broadcasts the denominator to every partition for free
    (PE cost is per-column, independent of M).  DVE reciprocal + multiply
    normalizes.  8.7x fewer denominator columns than a per-chunk ones-matmul.
  - PE instruction stream is organized as closed accumulation groups: all
    score matmuls for a (head, qblock) are single start|stop groups, then ONE
    uninterrupted PV accumulation chain.  Interleaving other matmuls into an
    open PSUM accumulation group costs ~80-120ns per transition on TRN2;
    closed-group transitions cost ~10-30ns.
"""

import numpy as np
import ml_dtypes

import concourse.bass as bass
import concourse.tile as tile
import concourse.mybir as mybir
from concourse import bacc
from concourse.bass import ts
from concourse.bass_utils import run_bass_kernel_spmd

B, S, H = 2, 2048, 2048
HEADS, HD = 16, 128
NCORES = 8
GH = 4                 # heads per core
GO = GH * HD           # 512 projection channels per core
P = 128
SB = 512               # token-block (free dim of most matmuls)
NSB = S // SB          # 4
HC = H // P            # 16 contraction chunks of the hidden dim
NKC = S // P           # 16 key-token chunks
SCALE = float(HD) ** -0.5

BF16 = mybir.dt.bfloat16
F32 = mybir.dt.float32
EXP = mybir.ActivationFunctionType.Exp

_built = {}


def _build():
    nc = bacc.Bacc(trn_type="TRN2")

    xt = nc.dram_tensor("xt", [H, S], BF16, kind="ExternalInput")
    wqt = nc.dram_tensor("wqt", [H, GO], BF16, kind="ExternalInput")
    wkt = nc.dram_tensor("wkt", [H, GO], BF16, kind="ExternalInput")
    wvt = nc.dram_tensor("wvt", [H, GO], BF16, kind="ExternalInput")
    wot = nc.dram_tensor("wot", [GO, H], BF16, kind="ExternalInput")
    cost = nc.dram_tensor("cost", [P, S], BF16, kind="ExternalInput")
    sint = nc.dram_tensor("sint", [P, S], BF16, kind="ExternalInput")
    # trimm[a, b] = -1e30 where b > a else 0; iden = identity.  The causal
    # mask is applied on the PE: psum[p, f] += trimm^T = -1e30 where p > f.
    trimm = nc.dram_tensor("trimm", [P, P], BF16, kind="ExternalInput")
    iden = nc.dram_tensor("iden", [P, P], BF16, kind="ExternalInput")
    yt = nc.dram_tensor("yt", [H, S], BF16, kind="ExternalOutput")

    xt_r = xt[:].rearrange("(hc p) s -> p hc s", p=P)
    yt_r = yt[:].rearrange("(t p) s -> p t s", p=P)

    with tile.TileContext(nc) as tc:
        with (
            tc.tile_pool(name="const", bufs=1) as const,
            tc.tile_pool(name="xstream", bufs=2) as xpool,
            tc.tile_pool(name="rope", bufs=2) as rpool,
            tc.tile_pool(name="attn", bufs=2) as apool,
            tc.tile_pool(name="den", bufs=2) as dpool,
            tc.tile_pool(name="yout", bufs=6) as ypool,
            tc.tile_pool(name="pacc", bufs=2, space="PSUM") as pacc,
            tc.tile_pool(name="pscore", bufs=2, space="PSUM") as pscore,
            tc.tile_pool(name="pout", bufs=2, space="PSUM") as pout,
        ):
            # ---- constants / persistent tensors ----
            # Startup DMAs fan out over three engine queues so the first
            # projection chain (which consumes xb0 + w_q chunk-by-chunk)
            # isn't serialized behind one queue's issue rate.
            xb0 = xpool.tile([P, HC, SB], BF16, tag="xb")
            w_q = const.tile([P, HC, GO], BF16, tag="wq")
            xt0 = xt_r[:, :, ts(0, SB)]
            wq_r = wqt[:].rearrange("(hc p) o -> p hc o", p=P)
            # the first two hc chunks go as single-chunk pieces so the first
            # projection chain can start sooner; the rest as pairs.
            pieces = [(0, 1), (1, 1), (2, 2), (4, 2), (6, 2), (8, 2), (10, 2),
                      (12, 2), (14, 2)]
            for i, (lo, n) in enumerate(pieces):
                nc.sync.dma_start(xb0[:, lo:lo + n, :], xt0[:, lo:lo + n, :])
                weng = nc.scalar if i % 2 == 0 else nc.gpsimd
                weng.dma_start(w_q[:, lo:lo + n, :], wq_r[:, lo:lo + n, :])
            w_k = const.tile([P, HC, GO], BF16, tag="wk")
            nc.sync.dma_start(w_k[:], wkt[:].rearrange("(hc p) o -> p hc o", p=P))
            cos_t = const.tile([P, S], BF16, tag="cos")
            nc.scalar.dma_start(cos_t[:], cost[:])
            sin_t = const.tile([P, S], BF16, tag="sin")
            nc.scalar.dma_start(sin_t[:], sint[:])
            tri_t = const.tile([P, P], BF16, tag="tri")
            nc.scalar.dma_start(tri_t[:], trimm[:])
            id_t = const.tile([P, P], BF16, tag="iden")
            nc.scalar.dma_start(id_t[:], iden[:])
            w_v = const.tile([P, HC, GO], BF16, tag="wv")
            nc.sync.dma_start(w_v[:], wvt[:].rearrange("(hc p) o -> p hc o", p=P))
            xbs = [xb0]
            xb1 = xpool.tile([P, HC, SB], BF16, tag="xb")
            nc.sync.dma_start(xb1[:], xt_r[:, :, ts(1, SB)])
            xbs.append(xb1)

            q_t = const.tile([P, GH, S], BF16, tag="qt")
            k_t = const.tile([P, GH, S], BF16, tag="kt")
            v_t = const.tile([P, NKC, GO], BF16, tag="vt")
            out_t = const.tile([P, GH, S], BF16, tag="ot")
            ones_t = const.tile([P, P], BF16, tag="ones")
            nc.gpsimd.memset(ones_t[:], 1.0)

            MM_NS = 0.4167e-3  # PE stream: us per column

            # ---- emission generators.  Every yield is a CLOSED PE group
            # boundary; the yielded value is the quantum's PE time (us). ----

            def proj_sb(sb, xb):
                """One token-block of Q/K (with RoPE) and V projections."""
                for w_t, dest in ((w_q, q_t), (w_k, k_t)):
                    for h in range(GH):
                        ps = pacc.tile([P, SB], F32, tag="pp")
                        for hc in range(HC):
                            nc.tensor.matmul(
                                ps[:], w_t[:, hc, ts(h, P)], xb[:, hc, :],
                                start=(hc == 0), stop=(hc == HC - 1),
                            )
                        raw = dest[:, h, ts(sb, SB)]
                        nc.scalar.copy(raw, ps[:])
                        # RoPE: rot = raw*cos + shift(raw)*sin_signed
                        tmp = rpool.tile([P, SB], BF16, tag="sh")
                        nc.sync.dma_start(tmp[0:64, :], raw[64:128, :])
                        nc.sync.dma_start(tmp[64:128, :], raw[0:64, :])
                        tmp2 = rpool.tile([P, SB], BF16, tag="sp")
                        nc.vector.tensor_mul(tmp2[:], tmp[:], sin_t[:, ts(sb, SB)])
                        nc.vector.tensor_mul(raw, raw, cos_t[:, ts(sb, SB)])
                        nc.vector.tensor_add(raw, raw, tmp2[:])
                        yield 3.41
                for j in range(SB // P):
                    ps = pacc.tile([P, GO], F32, tag="pp")
                    for hc in range(HC):
                        nc.tensor.matmul(
                            ps[:], xb[:, hc, ts(j, P)], w_v[:, hc, :],
                            start=(hc == 0), stop=(hc == HC - 1),
                        )
                    nc.scalar.copy(v_t[:, sb * (SB // P) + j, :], ps[:])
                    yield 3.41

            def attn_block(b):
                """Attention for one 512-query block.

                Per head: score matmuls as closed start|stop groups (chunk
                pairs share a 2-bank psum tile and one paired ACT exp), the
                denominator accumulating on DVE behind the exps, then one
                uninterrupted PV accumulation chain."""
                nchunks = 4 * (b + 1)
                noff = 4 * b
                prev = None  # (h, at, den1) awaiting PV emission

                def emit_pv(h, at, den1):
                    po = pout.tile([P, SB], F32, tag="po")
                    for c in range(nchunks):
                        j = c - noff
                        qlo = 128 * j if j > 0 else 0
                        nc.tensor.matmul(
                            po[:, qlo:], v_t[:, c, ts(h, P)], at[:, c, qlo:],
                            start=(c == 0), stop=(c == nchunks - 1),
                        )
                    # denominator: one 512-col all-ones matmul sums the DVE
                    # chunk-fold over the 128 key partitions AND broadcasts
                    # it to all partitions (cost is per-column, M-free).
                    # Emitted after the PV chain so the DVE fold has a full
                    # chain's slack before the PE reaches it.
                    pd = pacc.tile([P, SB], F32, tag="pp")
                    nc.tensor.matmul(pd[:], ones_t[:], den1, start=True, stop=True)
                    rec = dpool.tile([P, SB], F32, tag="rec")
                    nc.vector.reciprocal_approx_fast(rec[:], pd[:])
                    nc.vector.tensor_mul(out_t[:, h, ts(b, SB)], po[:], rec[:])

                for h in range(GH):
                    at = apool.tile([P, NKC, SB], BF16, tag="at")
                    den = dpool.tile([P, 2, SB], BF16, tag="den")
                    # off-diagonal chunk pairs
                    for i in range(noff // 2):
                        psc = pscore.tile([P, 2, SB], F32, tag="ps")
                        for m in (0, 1):
                            c = 2 * i + m
                            nc.tensor.matmul(
                                psc[:, m, :], k_t[:, h, ts(c, P)],
                                q_t[:, h, ts(b, SB)],
                                start=True, stop=True,
                            )
                        nc.scalar.activation(
                            at[:, 2 * i:2 * i + 2, :], psc[:, :, :], EXP,
                            scale=SCALE,
                        )
                        if i == 0:
                            nc.vector.tensor_copy(den[:], at[:, 0:2, :])
                        else:
                            nc.vector.tensor_add(
                                den[:], den[:], at[:, 2 * i:2 * i + 2, :]
                            )
                        yield 0.43
                    # diagonal chunks, two per psum pair tile, causal
                    # triangle accumulated on the PE inside the same group
                    for i in range(2):
                        psc = pscore.tile([P, 2, SB], F32, tag="ps")
                        for m in (0, 1):
                            c = noff + 2 * i + m
                            j = c - noff
                            qlo = 128 * j
                            n = SB - qlo
                            nc.tensor.matmul(
                                psc[:, m, 0:n], k_t[:, h, ts(c, P)],
                                q_t[:, h, b * SB + qlo:(b + 1) * SB],
                                start=True, stop=False,
                            )
                            nc.tensor.matmul(
                                psc[:, m, 0:128], tri_t[:], id_t[:],
                                start=False, stop=True,
                            )
                        for m in (0, 1):
                            c = noff + 2 * i + m
                            j = c - noff
                            qlo = 128 * j
                            n = SB - qlo
                            nc.scalar.activation(
                                at[:, c, qlo:], psc[:, m, 0:n], EXP,
                                scale=SCALE,
                            )
                            if b == 0 and c == 0:
                                nc.vector.tensor_copy(den[:, 0, :], at[:, 0, :])
                            elif b == 0:
                                nc.vector.tensor_add(
                                    den[:, 0, qlo:], den[:, 0, qlo:],
                                    at[:, c, qlo:],
                                )
                            else:
                                nc.vector.tensor_add(
                                    den[:, 0, qlo:], den[:, 0, qlo:],
                                    at[:, c, qlo:],
                                )
                        yield 0.37
                    # fold the two accumulator halves on DVE; this overlaps
                    # the next head's scores, so the PE-side den matmul in
                    # emit_pv never waits on it.
                    if b == 0:
                        den1 = den[:, 0, :]
                    else:
                        nc.vector.tensor_add(
                            den[:, 0, :], den[:, 0, :], den[:, 1, :]
                        )
                        den1 = den[:, 0, :]
                    if prev is not None:
                        emit_pv(*prev)
                        yield (1280 + 2048 * b) * MM_NS
                    prev = (h, at, den1)
                emit_pv(*prev)
                yield (1280 + 2048 * b) * MM_NS

            NT = H // P

            def outproj_block(ob, tail=False):
                """Output projection of one query block; yields per closed
                4-matmul chain.  While interleaved with attention the
                PSUM->SBUF staging copies stay on DVE (keeping ACT free for
                the latency-critical exps); in the tail they alternate."""
                for nt in range(NT):
                    pyt = pacc.tile([P, SB], F32, tag="pp")
                    for oc in range(GH):
                        nc.tensor.matmul(
                            pyt[:], w_o[:, oc, ts(nt, P)],
                            out_t[:, oc, ts(ob, SB)],
                            start=(oc == 0), stop=(oc == GH - 1),
                        )
                    ysb = ypool.tile([P, SB], BF16, tag="ysb")
                    if tail and nt % 2 == 1:
                        nc.scalar.copy(ysb[:], pyt[:])
                    else:
                        nc.vector.tensor_copy(ysb[:], pyt[:])
                    nc.sync.dma_start(yt_r[:, nt, ts(ob, SB)], ysb[:])
                    yield 0.85

            def drain(gen):
                for _ in gen:
                    pass

            def chain2(*gens):
                for g in gens:
                    yield from g

            def interleave(primary, filler, ratio, drain_rest=True):
                """Emit primary; between its quanta emit filler quanta so
                filler-PE-time ~= ratio * primary-PE-time."""
                acc = 0.0
                done = False
                for wp in primary:
                    acc += ratio * (wp or 1.0)
                    while not done and acc > 0:
                        wf = next(filler, None)
                        if wf is None:
                            done = True
                        else:
                            acc -= wf or 1.0
                if drain_rest and not done:
                    drain(filler)

            # ---- drive ----
            drain(proj_sb(0, xbs[0]))

            def proj_one(sb):
                # prefetch the NEXT x block after the first chain of this
                # one, so its (single-queue) DMA lands well before use.
                first = True
                for w in proj_sb(sb, xbs[sb]):
                    yield w
                    if first:
                        first = False
                        if sb + 1 < NSB and len(xbs) == sb + 1:
                            xbn = xpool.tile([P, HC, SB], BF16, tag="xb")
                            nc.sync.dma_start(xbn[:], xt_r[:, :, ts(sb + 1, SB)])
                            xbs.append(xbn)

            interleave(proj_one(1), attn_block(0), 5.2 / 41.0)
            interleave(proj_one(2), attn_block(1), 12.1 / 41.0)
            # Wo reuses an x-stream slot (same bytes); loaded once proj(2)
            # has consumed xb2, well before outproj(0) needs it.
            w_o = xpool.tile([P, GH, H], BF16, tag="xb")
            nc.sync.dma_start(w_o[:], wot[:].rearrange("(oc p) n -> p oc n", p=P))
            interleave(proj_one(3), attn_block(2), 18.8 / 41.0)
            interleave(
                attn_block(3),
                chain2(outproj_block(0), outproj_block(1), outproj_block(2)),
                1.6,
            )
            drain(outproj_block(NSB - 1, tail=True))

    nc.compile()
    return nc


def _get_nc():
    if "nc" not in _built:
        _built["nc"] = _build()
    return _built["nc"]


def _host_inputs(x, Wq, Wk, Wv, Wo):
    bf = ml_dtypes.bfloat16
    inv = 1.0 / (10000.0 ** (np.arange(0, HD, 2, dtype=np.float64) / HD))
    t = np.arange(S, dtype=np.float64)
    fr = np.outer(t, inv)                       # [S, 64]
    cos = np.cos(fr)
    sin = np.sin(fr)
    cosT = np.concatenate([cos, cos], axis=1).T.astype(bf)      # [128, S]
    sinT = np.concatenate([-sin, sin], axis=1).T.astype(bf)     # signed
    a = np.arange(P)
    trimm = np.where(a[None, :] > a[:, None], -1e30, 0.0).astype(bf)
    iden = np.eye(P).astype(bf)

    in_maps = []
    for core in range(NCORES):
        b, g = divmod(core, GH)
        sl = slice(GO * g, GO * (g + 1))
        in_maps.append({
            "xt": np.ascontiguousarray(x[b].T).astype(bf),
            "wqt": np.ascontiguousarray(Wq[sl, :].T).astype(bf),
            "wkt": np.ascontiguousarray(Wk[sl, :].T).astype(bf),
            "wvt": np.ascontiguousarray(Wv[sl, :].T).astype(bf),
            "wot": np.ascontiguousarray(Wo[:, sl].T).astype(bf),
            "cost": cosT.copy(),
            "sint": sinT.copy(),
            "trimm": trimm.copy(),
            "iden": iden.copy(),
        })
    return in_maps


def kernel(x, Wq, Wk, Wv, Wo, _trace=False):
    x = np.asarray(x, dtype=np.float32)
    Wq = np.asarray(Wq, dtype=np.float32)
    Wk = np.asarray(Wk, dtype=np.float32)
    Wv = np.asarray(Wv, dtype=np.float32)
    Wo = np.asarray(Wo, dtype=np.float32)

    nc = _get_nc()
    in_maps = _host_inputs(x, Wq, Wk, Wv, Wo)
    res = run_bass_kernel_spmd(
        nc, in_maps, core_ids=list(range(NCORES)), trace=_trace
    )
    if _trace:
        _built["last_result"] = res

    y = np.zeros((B, S, H), dtype=np.float32)
    for core in range(NCORES):
        b = core // GH
        y[b] += res.results[core]["yt"].T.astype(np.float32)
    return y
